# revision 33
# baseline (speedup 1.0000x reference)
"""Trainium2 Bass kernel for nn_AttentionBlock (B=8, C=1024, L=1024, H=16, G=32).

Data-parallel over batch: one sample per NeuronCore, no collectives.
fp8e4 (e4m3, TRN max 240) DoubleRow matmuls carry the heavy GEMMs: each
DoubleRow instruction contracts 256 rows (two 128-deep k-subtiles packed
into the PE array) per moving column, halving PE time vs bf16.

Per core:
  1. GroupNorm(32): per-channel sum (DVE) + sumsq (ACT Square-accum),
     cross-partition group reduce/broadcast via tiny fp32 matmuls,
     rsqrt via Ln/Exp (single ACT table set).  Apply produces
       - xn8: fp8 x_norm in DoubleRow pair-interleaved layout
         [128, 2, L] x 4 tiles (moving operand for qkv projections),
       - xn':  fp32 x_norm + proj_bias_eff (residual + epilogue bias,
         computed on the Pool engine to keep DVE free).
     Weights are host-scaled x32 (power of two) and clipped to +-240.
  2. v^T via operand swap (stationary = xn8 chunk), DoubleRow, directly
     in [128, 2(s-parity), 16 heads, 65] fp8 tiles; column 64 holds the
     constant 32 so the attention mm2 emits the softmax denominator.
  3. Attention, ACT-bound: per s-chunk one [128, 2(head), 512] PSUM score
     tile, one 1024-wide ACT exp (scale=1/8, bias=-2) writing fp8 into
     the DoubleRow-interleaved ex8 tile, then one DoubleRow mm2 per head
     per s-chunk-pair.  Softmax needs no max subtraction (|z|/8 <= ~6.1;
     bias -2 keeps exp <= ~60 < 240).  Normalize: DVE reciprocal of the
     denominator row straight out of PSUM, Pool-engine partition
     broadcast, one DVE multiply into the fp8 proj operand.  The next
     pair's q/k projection (DoubleRow) is interleaved one chunk per
     s-step so the PE never drains at pair boundaries.
  4. proj (DoubleRow) + (xn + bias_eff) residual epilogue, DMA out on
     two queues.  v-bias is folded into proj bias on the host.
"""

import numpy as np
import ml_dtypes

import concourse.bass as bass
import concourse.bacc as bacc
import concourse.tile as tile
from concourse import mybir
from concourse.bass_utils import run_bass_kernel_spmd

F32 = mybir.dt.float32
BF16 = mybir.dt.bfloat16
E4 = mybir.dt.float8e4
DR = mybir.MatmulPerfMode.DoubleRow

B, C, L, H = 8, 1024, 1024, 16
GROUPS = 32
CH = C // H          # 64 per-head channels
EPS = 1e-5
NT = C // 128        # 8 channel tiles
LT = L // 512        # 2 free-dim chunks of 512
PAIRS = H // 2       # 8 head pairs
KK = 4               # DoubleRow contraction steps (4 x 256 = 1024)
SCP = 4              # s-chunk pairs per 1024 positions
WS = 32.0            # host weight scale (power of 2)
EXP_BIAS = -2.0      # exp(z/8 + EXP_BIAS): max ~exp(4.1)=60 << 240


DEBUG_DUMPS = False


def declare_params(nc):
    p = {}
    if DEBUG_DUMPS:
        p["dbg_xn8"] = nc.declare_dram_parameter(
            "dbg_xn8", [KK, 128, 2, L], E4, isOutput=True)
        p["dbg_q0"] = nc.declare_dram_parameter(
            "dbg_q0", [128, L], BF16, isOutput=True)
        p["dbg_k0"] = nc.declare_dram_parameter(
            "dbg_k0", [128, L], BF16, isOutput=True)
        p["dbg_vT0"] = nc.declare_dram_parameter(
            "dbg_vT0", [SCP, 128, 2, H, CH + 1], E4, isOutput=True)
        p["dbg_ab8"] = nc.declare_dram_parameter(
            "dbg_ab8", [KK, 128, 2, L], E4, isOutput=True)
        p["dbg_m1"] = nc.declare_dram_parameter(
            "dbg_m1", [128, 2, 512], F32, isOutput=True)
        p["dbg_ex8"] = nc.declare_dram_parameter(
            "dbg_ex8", [128, 2, 2, 512], E4, isOutput=True)
        p["dbg_ps2"] = nc.declare_dram_parameter(
            "dbg_ps2", [2, CH + 1, 512], F32, isOutput=True)
    p["x"] = nc.declare_dram_parameter("x", [C, L], F32, isOutput=False)
    p["q_w8"] = nc.declare_dram_parameter("q_w8", [128, PAIRS, KK, 2, 128],
                                          E4, isOutput=False)
    p["k_w8"] = nc.declare_dram_parameter("k_w8", [128, PAIRS, KK, 2, 128],
                                          E4, isOutput=False)
    p["v_w8"] = nc.declare_dram_parameter("v_w8", [128, LT, KK, 2, 512],
                                          E4, isOutput=False)
    p["p_w8"] = nc.declare_dram_parameter("p_w8", [128, NT, KK, 2, 128],
                                          E4, isOutput=False)
    p["q_b"] = nc.declare_dram_parameter("q_b", [128, PAIRS], F32, isOutput=False)
    p["k_b"] = nc.declare_dram_parameter("k_b", [128, PAIRS], F32, isOutput=False)
    p["proj_beff"] = nc.declare_dram_parameter("proj_beff", [128, NT], F32,
                                               isOutput=False)
    p["norm_w_c"] = nc.declare_dram_parameter("norm_w_c", [128, NT], F32,
                                              isOutput=False)
    p["norm_b_c"] = nc.declare_dram_parameter("norm_b_c", [128, NT], F32,
                                              isOutput=False)
    p["A_grp"] = nc.declare_dram_parameter("A_grp", [128, 4], F32,
                                           isOutput=False)
    p["A2T"] = nc.declare_dram_parameter("A2T", [4, 128], F32, isOutput=False)
    p["out"] = nc.declare_dram_parameter("out", [C, L], F32, isOutput=True)
    return p


def emit(nc, tc, ctx, params, out_handle=None):
    from contextlib import ExitStack

    x_d = params["x"]
    out_d = params["out"] if out_handle is None else out_handle
    x_ap, out_ap = x_d.ap(), out_d.ap()
    qw8, kw8 = params["q_w8"].ap(), params["k_w8"].ap()
    vw8, pw8 = params["v_w8"].ap(), params["p_w8"].ap()

    # ---- persistent pools --------------------------------------------
    consts = ctx.enter_context(tc.tile_pool(name="consts", bufs=1))
    xn_p = ctx.enter_context(tc.tile_pool(name="xn", bufs=NT))
    xn8_p = ctx.enter_context(tc.tile_pool(name="xn8", bufs=KK))
    vT_p = ctx.enter_context(tc.tile_pool(name="vT", bufs=SCP))
    ab8_p = ctx.enter_context(tc.tile_pool(name="ab8", bufs=LT * KK))
    # all 16 q/k tiles stay live through both tcn passes
    qk_p = ctx.enter_context(tc.tile_pool(name="qk", bufs=2 * PAIRS + 1))
    qkw_p = ctx.enter_context(tc.tile_pool(name="qkw", bufs=4))
    pw_p = ctx.enter_context(tc.tile_pool(name="pw", bufs=NT))
    # one shared 2-bank PSUM pool for the vT / qk-projection / proj
    # accumulators (never more than two of those streams active at once)
    aux_ps = ctx.enter_context(
        tc.tile_pool(name="auxps", bufs=1, space=bass.MemorySpace.PSUM))

    xn = []    # 8 x [128, L] f32: x_norm + proj_beff (residual + bias)
    xn8 = []   # 4 x [128, 2, L] e4m3: DoubleRow moving operand

    # ================= Phase 1: GroupNorm =============================
    with ExitStack() as ph1:
        xp = ph1.enter_context(tc.tile_pool(name="xp", bufs=NT // 2))
        scr_p = ph1.enter_context(tc.tile_pool(name="scr", bufs=2))
        gn_p = ph1.enter_context(tc.tile_pool(name="gn", bufs=1))
        gnps = ph1.enter_context(
            tc.tile_pool(name="gnps", bufs=2, space=bass.MemorySpace.PSUM))

        # x first: 4 wide DMAs (2 channel-tiles each) over 3 queues to
        # minimize per-descriptor round trips; consts/weights queue behind
        xt = []
        x_engs = [nc.sync, nc.gpsimd, nc.scalar]
        xr = x_ap.rearrange("(g p) l -> p g l", p=128)
        for d in range(4):
            big = xp.tile([128, 2, L], F32, tag="x_t", name="x_t")
            x_engs[d % 3].dma_start(out=big, in_=xr[:, 2 * d:2 * d + 2, :])
            xt.append(big[:, 0, :])
            xt.append(big[:, 1, :])

        def load_const(dram_name, shape, tag):
            t = consts.tile(shape, F32, tag=tag, name=tag)
            nc.scalar.dma_start(out=t, in_=params[dram_name].ap())
            return t

        ag_sb = load_const("A_grp", [128, 4], "ag")
        a2_sb = load_const("A2T", [4, 128], "a2")
        qb_sb = load_const("q_b", [128, PAIRS], "qb")
        kb_sb = load_const("k_b", [128, PAIRS], "kb")
        pb_sb = load_const("proj_beff", [128, NT], "pb")
        nw_sb = load_const("norm_w_c", [128, NT], "nw")
        nb_sb = load_const("norm_b_c", [128, NT], "nb")
        eps_sb = consts.tile([4, 1], F32, tag="eps", name="eps")
        nc.vector.memset(eps_sb, EPS)
        ebias_sb = consts.tile([128, 1], F32, tag="ebias", name="ebias")
        nc.vector.memset(ebias_sb, EXP_BIAS)
        ones_f = consts.tile([128, 2 * H], F32, tag="ones_f", name="ones_f")
        nc.vector.memset(ones_f, 1.0)

        # v weights (needed in phase 2), then proj weights (phase 4)
        vw_sb = []
        for n in range(LT):
            t = consts.tile([128, KK, 2, 512], E4, tag=f"vw{n}", name="vw_t")
            eng = nc.sync if n % 2 == 0 else nc.gpsimd
            eng.dma_start(out=t, in_=vw8[:, n])
            vw_sb.append(t)
        pw_sb = []
        for m in range(NT):
            t = pw_p.tile([128, KK, 2, 128], E4, tag="pw_t", name="pw_t")
            eng = nc.sync if m % 2 == 0 else nc.gpsimd
            eng.dma_start(out=t, in_=pw8[:, m])
            pw_sb.append(t)

        warm = gn_p.tile([4, 1], F32, tag="warm", name="warm")
        nc.scalar.activation(out=warm, in_=eps_sb,
                             func=mybir.ActivationFunctionType.Ln,
                             bias=eps_sb, scale=1.0)

        stats = gn_p.tile([128, 2 * NT], F32, tag="stats", name="stats")
        for t in range(NT):
            nc.vector.reduce_sum(
                out=stats[:, t:t + 1], in_=xt[t], axis=mybir.AxisListType.X)
            scr = scr_p.tile([128, L], F32, tag="scr", name="scr")
            nc.scalar.activation(
                out=scr, in_=xt[t],
                func=mybir.ActivationFunctionType.Square,
                accum_out=stats[:, NT + t:NT + t + 1])

        gps = gnps.tile([4, 2 * NT], F32, tag="gps", name="gps")
        nc.tensor.matmul(gps, ag_sb, stats)

        mv16 = gn_p.tile([4, 2 * NT], F32, tag="mv16", name="mv16")
        inv_n = 1.0 / (32 * L)
        nc.vector.tensor_scalar_mul(out=mv16[:, 0:NT], in0=gps[:, 0:NT],
                                    scalar1=inv_n)
        e2 = gn_p.tile([4, NT], F32, tag="e2", name="e2")
        nc.vector.tensor_scalar_mul(out=e2, in0=gps[:, NT:2 * NT],
                                    scalar1=inv_n)
        m2 = gn_p.tile([4, NT], F32, tag="m2", name="m2")
        nc.vector.tensor_tensor(out=m2, in0=mv16[:, 0:NT], in1=mv16[:, 0:NT],
                                op=mybir.AluOpType.mult)
        var = gn_p.tile([4, NT], F32, tag="var", name="var")
        nc.vector.tensor_tensor(out=var, in0=e2, in1=m2,
                                op=mybir.AluOpType.subtract)
        lnv = gn_p.tile([4, NT], F32, tag="lnv", name="lnv")
        nc.scalar.activation(out=lnv, in_=var,
                             func=mybir.ActivationFunctionType.Ln,
                             bias=eps_sb, scale=1.0)
        nc.scalar.activation(out=mv16[:, NT:2 * NT], in_=lnv,
                             func=mybir.ActivationFunctionType.Exp,
                             scale=-0.5)

        bc = gnps.tile([128, 2 * NT], F32, tag="bc", name="bc")
        nc.tensor.matmul(bc, a2_sb, mv16)

        scale_sb = gn_p.tile([128, NT], F32, tag="scale", name="scale")
        nc.vector.tensor_tensor(out=scale_sb, in0=nw_sb, in1=bc[:, NT:2 * NT],
                                op=mybir.AluOpType.mult)
        tmp = gn_p.tile([128, NT], F32, tag="tmp", name="tmp")
        nc.vector.tensor_tensor(out=tmp, in0=bc[:, 0:NT], in1=scale_sb,
                                op=mybir.AluOpType.mult)
        bias_sb = gn_p.tile([128, NT], F32, tag="bias", name="bias")
        nc.vector.tensor_tensor(out=bias_sb, in0=nb_sb, in1=tmp,
                                op=mybir.AluOpType.subtract)
        # residual copy also carries the proj bias: bias2 = bias + proj_beff
        bias2_sb = gn_p.tile([128, NT], F32, tag="bias2", name="bias2")
        nc.vector.tensor_tensor(out=bias2_sb, in0=bias_sb, in1=pb_sb,
                                op=mybir.AluOpType.add)

        for kk in range(KK):
            x8 = xn8_p.tile([128, 2, L], E4, tag="xn8_t", name="xn8_t")
            xn8.append(x8)
        for t in range(NT):
            nc.scalar.activation(
                out=xn8[t // 2][:, t % 2, :], in_=xt[t],
                func=mybir.ActivationFunctionType.Identity,
                scale=scale_sb[:, t:t + 1], bias=bias_sb[:, t:t + 1])
        for t in range(NT):
            xnt = xn_p.tile([128, L], F32, tag="xn_t", name="xn_t")
            nc.gpsimd.tensor_scalar(
                out=xnt, in0=xt[t],
                scalar1=scale_sb[:, t:t + 1], scalar2=bias2_sb[:, t:t + 1],
                op0=mybir.AluOpType.mult, op1=mybir.AluOpType.add)
            xn.append(xnt)

    # ================= Phase 2: v^T ===================================
    # vT8[scp][s, i, h, c] = v[64h+c, s-pos of chunk 2scp+i]; col 64 = 32.0
    qk_res = {}

    def qk_gen(j):
        """Emit pair j's q/k projection (DoubleRow) in small chunks."""
        wts = {}
        for name, w_ap in (("q", qw8), ("k", kw8)):
            wt = qkw_p.tile([128, KK, 2, 128], E4, tag="qkw_t", name="qkw_t")
            nc.sync.dma_start(out=wt, in_=w_ap[:, j])
            wts[name] = wt
        yield
        for name, b_sb in (("q", qb_sb), ("k", kb_sb)):
            wt = wts[name]
            dst = qk_p.tile([128, L], BF16, tag=f"{name}_j", name=f"{name}_j")
            for n in range(LT):
                acc = aux_ps.tile([128, 512], F32, tag="aux", name="aux")
                for kk in range(KK):
                    nc.tensor.matmul(
                        acc, wt[:, kk], xn8[kk][:, :, n * 512:(n + 1) * 512],
                        perf_mode=DR, start=(kk == 0), stop=(kk == KK - 1))
                    if kk == 1:
                        yield
                nc.vector.tensor_scalar_add(
                    out=dst[:, n * 512:(n + 1) * 512], in0=acc,
                    scalar1=b_sb[:, j:j + 1])
                yield
            qk_res.setdefault(j, {})[name] = dst

    # v^T is emitted as a generator interleaved into the first attention
    # pairs: per (lc-pair, n) one 2-bank accumulator group, so vT8[scp]
    # completes after its two (scp, n) groups and mm2 unblocks per-scp.
    vT8 = []
    for scp in range(SCP):
        vt = vT_p.tile([128, 2, H, CH + 1], E4, tag="vT_t", name="vT_t")
        nc.vector.tensor_copy(
            out=vt[:, :, :, CH],
            in_=ones_f.rearrange("p (i h) -> p i h", i=2))
        vT8.append(vt)

    def vt_gen():
        for lc in range(2 * SCP):
            for n in range(LT):
                acc = aux_ps.tile([128, 512], F32, tag="aux", name="aux")
                for kk in range(KK):
                    nc.tensor.matmul(
                        acc,
                        xn8[kk][:, :, lc * 128:(lc + 1) * 128],
                        vw_sb[n][:, kk],
                        perf_mode=DR, start=(kk == 0), stop=(kk == KK - 1))
                    if kk < KK - 1:
                        yield
                nc.vector.tensor_scalar_mul(
                    out=vT8[lc // 2][:, lc % 2, n * 8:(n + 1) * 8, 0:CH],
                    in0=acc.rearrange("p (h c) -> p h c", c=CH),
                    scalar1=1.0 / WS)
                yield

    # ============ Phase 3: attention (tcn-outer) ======================
    # tcn0 interleaves the next pair's q/k projection; tcn1 interleaves
    # the proj matmuls for the already-complete tcn0 half of ab8, so the
    # epilogue tail is only the tcn1 half of proj.
    ab8 = [[ab8_p.tile([128, 2, 512], E4, tag="ab8_t", name="ab8_t")
            for _ in range(KK)] for _ in range(LT)]

    def proj_emit(n, out_p, use_m1=False):
        for m in range(NT):
            if use_m1:
                mt = m1_p.tile([128, 2, 512], F32, tag="m1", name="m1")
                acc = mt[:, 0, :]
            else:
                acc = aux_ps.tile([128, 512], F32, tag="aux", name="aux")
            for kk in range(KK):
                nc.tensor.matmul(
                    acc, pw_sb[m][:, kk],
                    ab8[n][kk],
                    perf_mode=DR, start=(kk == 0), stop=(kk == KK - 1))
                if kk == 1:
                    yield
            o_sb = out_p.tile([128, 512], F32, tag="o_sb", name="o_sb")
            nc.vector.scalar_tensor_tensor(
                out=o_sb, in0=acc, scalar=1.0 / WS,
                in1=xn[m][:, n * 512:(n + 1) * 512],
                op0=mybir.AluOpType.mult, op1=mybir.AluOpType.add)
            eng = nc.sync if m % 2 == 0 else nc.gpsimd
            eng.dma_start(
                out=out_ap[m * 128:(m + 1) * 128, n * 512:(n + 1) * 512],
                in_=o_sb)
            yield

    with ExitStack() as ph3:
        ex_p = ph3.enter_context(tc.tile_pool(name="exp", bufs=4))
        rc_p = ph3.enter_context(tc.tile_pool(name="rcp", bufs=4))
        out_p = ph3.enter_context(tc.tile_pool(name="outp", bufs=4))
        m1_p = ph3.enter_context(
            tc.tile_pool(name="m1p", bufs=2, space=bass.MemorySpace.PSUM))
        ps2_p = ph3.enter_context(
            tc.tile_pool(name="ps2p", bufs=3, space=bass.MemorySpace.PSUM))

        for _ in qk_gen(0):
            pass

        vtg = vt_gen()
        vt_done = 0
        proj0 = None
        for tcn in range(LT):
            if tcn == 1:
                proj0 = proj_emit(0, out_p)
            for j in range(PAIRS):
                if tcn == 0:
                    nxt = qk_gen(j + 1) if j + 1 < PAIRS else None
                else:
                    nxt = proj0
                q_j = qk_res[j]["q"]
                k_j = qk_res[j]["k"]

                ps2 = [ps2_p.tile([CH + 1, 512], F32, tag="ps2", name="ps2")
                       for _ in range(2)]
                for scp in range(SCP):
                    ex8 = ex_p.tile([128, 2, 2, 512], E4, tag="ex8",
                                    name="ex8")
                    for i in range(2):
                        sc = 2 * scp + i
                        m1 = m1_p.tile([128, 2, 512], F32, tag="m1",
                                       name="m1")
                        for par in range(2):
                            base = CH * par
                            nc.tensor.matmul(
                                m1[:, par, :],
                                k_j[base:base + CH,
                                    sc * 128:(sc + 1) * 128],
                                q_j[base:base + CH,
                                    tcn * 512:(tcn + 1) * 512])
                        nc.scalar.activation(
                            out=ex8[:, i, :, :], in_=m1,
                            func=mybir.ActivationFunctionType.Exp,
                            scale=0.125 / (WS * WS), bias=ebias_sb)
                    if vtg is not None:
                        # vT8[scp] groups must be fully emitted before the
                        # first pair's mm2 that reads them
                        need = (2 * scp + 2) * 2 * KK
                        while vt_done < need:
                            next(vtg, None)
                            vt_done += 1
                        if vt_done >= 4 * KK * SCP:
                            vtg = None
                    for par in range(2):
                        nc.tensor.matmul(
                            ps2[par], vT8[scp][:, :, 2 * j + par, :],
                            ex8[:, :, par, :],
                            perf_mode=DR,
                            start=(scp == 0), stop=(scp == SCP - 1))
                    if nxt is not None:
                        next(nxt, None)
                        if tcn == 1:
                            next(nxt, None)
                # normalize: a = a_raw * (1 / S); S sits in psum row 64.
                # Order matters on HW: the partition_broadcast (gpsimd)
                # input must come from a plain DVE copy, and the custom-DVE
                # reciprocal output must be consumed by DVE only.
                for par in range(2):
                    base = CH * par
                    s_sb = rc_p.tile([1, 512], F32, tag="s_sb", name="s_sb")
                    nc.vector.tensor_copy(out=s_sb,
                                          in_=ps2[par][CH:CH + 1, :])
                    sbb = rc_p.tile([CH, 512], F32, tag="sbb", name="sbb")
                    nc.gpsimd.partition_broadcast(sbb, s_sb, channels=CH)
                    rc64 = rc_p.tile([CH, 512], F32, tag="rc64", name="rc64")
                    nc.vector.reciprocal_approx_fast(out=rc64, in_=sbb)
                    nc.vector.tensor_tensor(
                        out=ab8[tcn][j // 2][base:base + CH, j % 2, :],
                        in0=ps2[par][0:CH, :], in1=rc64,
                        op=mybir.AluOpType.mult)
                if tcn == 0 and nxt is not None:
                    for _ in nxt:
                        pass
        # ======== proj tail: drain n=0 remainder, then n=1 ============
        if proj0 is not None:
            for _ in proj0:
                pass
        for _ in proj_emit(1, out_p, use_m1=True):
            pass


_CACHED = {}


def build_program(repeats=1):
    key = ("nc", repeats)
    if key in _CACHED:
        return _CACHED[key]
    from contextlib import ExitStack

    nc = bacc.Bacc("TRN2", target_bir_lowering=False, debug=False)
    with tile.TileContext(nc) as tc:
        params = declare_params(nc)
        for rep in range(repeats):
            out_h = None
            if rep > 0:
                out_h = nc.dram_tensor(f"out_scratch{rep}", [C, L], F32)
            with ExitStack() as ctx:
                emit(nc, tc, ctx, params, out_h)
    nc.compile()
    _CACHED[key] = nc
    return nc


def to_e4(a):
    return np.clip(np.asarray(a, np.float32), -240, 240).astype(
        ml_dtypes.float8_e4m3)


def host_pack(norm_w, norm_b, qkv_w, qkv_b, proj_w, proj_b):
    """Precompute packed weight layouts (all plain numpy)."""
    f = np.float32
    qkv_w = np.asarray(qkv_w, f)
    qkv_b = np.asarray(qkv_b, f)
    proj_w = np.asarray(proj_w, f)
    proj_b = np.asarray(proj_b, f)

    idx_q = np.empty(C, np.int64)
    idx_k = np.empty(C, np.int64)
    for j in range(PAIRS):
        for p in range(128):
            h = 2 * j + p // CH
            i = p % CH
            idx_q[j * 128 + p] = 192 * h + i
            idx_k[j * 128 + p] = 192 * h + CH + i
    idx_v = np.empty(C, np.int64)
    for h in range(H):
        idx_v[CH * h:CH * (h + 1)] = 192 * h + 2 * CH + np.arange(CH)

    def pack_w(w, out_grp, out_sz):
        # w: [1024 out-rows, 1024 in-ch] -> [128, out_grp, KK, 2, out_sz]
        a = (WS * w).reshape(out_grp, out_sz, KK, 2, 128)
        return to_e4(np.ascontiguousarray(a.transpose(4, 0, 2, 3, 1)))

    q_w8 = pack_w(qkv_w[idx_q, :], PAIRS, 128)
    k_w8 = pack_w(qkv_w[idx_k, :], PAIRS, 128)
    v_w8 = pack_w(qkv_w[idx_v, :], LT, 512)
    p_w8 = pack_w(proj_w, NT, 128)

    # q/k are kept x32-scaled (psum + 32*bias); exp scale divides by 32^2
    q_b = np.ascontiguousarray(WS * qkv_b[idx_q].reshape(PAIRS, 128).T)
    k_b = np.ascontiguousarray(WS * qkv_b[idx_k].reshape(PAIRS, 128).T)
    pbe = proj_b + proj_w @ qkv_b[idx_v]
    proj_beff = np.ascontiguousarray(pbe.astype(f).reshape(NT, 128).T)

    norm_w_c = np.ascontiguousarray(np.asarray(norm_w, f).reshape(NT, 128).T)
    norm_b_c = np.ascontiguousarray(np.asarray(norm_b, f).reshape(NT, 128).T)

    pp = np.arange(128)
    A_grp = (pp[:, None] // 32 == np.arange(4)[None, :]).astype(f)
    A2T = np.ascontiguousarray(A_grp.T)

    return dict(
        q_w8=q_w8, k_w8=k_w8, v_w8=v_w8, p_w8=p_w8,
        q_b=q_b, k_b=k_b, proj_beff=proj_beff,
        norm_w_c=norm_w_c, norm_b_c=norm_b_c, A_grp=A_grp, A2T=A2T,
    )


def kernel(x, norm_w, norm_b, qkv_w, qkv_b, proj_w, proj_b, _trace=False):
    x = np.asarray(x, np.float32)
    shared = host_pack(norm_w, norm_b, qkv_w, qkv_b, proj_w, proj_b)
    nc = build_program()
    in_maps = [dict(shared, x=np.ascontiguousarray(x[i])) for i in range(B)]
    res = run_bass_kernel_spmd(nc, in_maps, list(range(B)), trace=_trace)
    out = np.stack([res.results[i]["out"] for i in range(B)], axis=0)
    if _trace:
        kernel._last_results = res
    return out.astype(np.float32)


# revision 34
# speedup vs baseline: 1.2680x; 1.2680x over previous
"""Trainium2 Bass kernel for nn_AttentionBlock (B=8, C=1024, L=1024, H=16, G=32).

Data-parallel over batch: one sample per NeuronCore, no collectives.
fp8e4 (e4m3, TRN max 240) DoubleRow matmuls carry the heavy GEMMs: each
DoubleRow instruction contracts 256 rows (two 128-deep k-subtiles packed
into the PE array) per moving column, halving PE time vs bf16.

Per core:
  1. GroupNorm(32): per-channel sum (DVE) + sumsq (ACT Square-accum),
     cross-partition group reduce/broadcast via tiny fp32 matmuls,
     rsqrt via Ln/Exp (single ACT table set).  Apply produces
       - xn8: fp8 x_norm in DoubleRow pair-interleaved layout
         [128, 2, L] x 4 tiles (moving operand for qkv projections),
       - xn':  fp32 x_norm + proj_bias_eff (residual + epilogue bias,
         computed on the Pool engine to keep DVE free).
     Weights are host-scaled x32 (power of two) and clipped to +-240.
  2. v^T via operand swap (stationary = xn8 chunk), DoubleRow, directly
     in [128, 2(s-parity), 16 heads, 65] fp8 tiles; column 64 holds the
     constant 32 so the attention mm2 emits the softmax denominator.
  3. Attention, ACT-bound: per s-chunk one [128, 2(head), 512] PSUM score
     tile, one 1024-wide ACT exp (scale=1/8, bias=-2) writing fp8 into
     the DoubleRow-interleaved ex8 tile, then one DoubleRow mm2 per head
     per s-chunk-pair.  Softmax needs no max subtraction (|z|/8 <= ~6.1;
     bias -2 keeps exp <= ~60 < 240).  Normalize: DVE reciprocal of the
     denominator row straight out of PSUM, Pool-engine partition
     broadcast, one DVE multiply into the fp8 proj operand.  The next
     pair's q/k projection (DoubleRow) is interleaved one chunk per
     s-step so the PE never drains at pair boundaries.
  4. proj (DoubleRow) + (xn + bias_eff) residual epilogue, DMA out on
     two queues.  v-bias is folded into proj bias on the host.
"""

import numpy as np
import ml_dtypes

import concourse.bass as bass
import concourse.bacc as bacc
import concourse.tile as tile
from concourse import mybir
from concourse.bass_utils import run_bass_kernel_spmd

F32 = mybir.dt.float32
BF16 = mybir.dt.bfloat16
E4 = mybir.dt.float8e4
DR = mybir.MatmulPerfMode.DoubleRow

B, C, L, H = 8, 1024, 1024, 16
GROUPS = 32
CH = C // H          # 64 per-head channels
EPS = 1e-5
NT = C // 128        # 8 channel tiles
LT = L // 512        # 2 free-dim chunks of 512
PAIRS = H // 2       # 8 head pairs
KK = 4               # DoubleRow contraction steps (4 x 256 = 1024)
SCP = 4              # s-chunk pairs per 1024 positions
WS = 32.0            # host weight scale (power of 2)
EXP_BIAS = -2.0      # exp(z/8 + EXP_BIAS): max ~exp(4.1)=60 << 240


DEBUG_DUMPS = False


def declare_params(nc):
    p = {}
    if DEBUG_DUMPS:
        p["dbg_xn8"] = nc.declare_dram_parameter(
            "dbg_xn8", [KK, 128, 2, L], E4, isOutput=True)
        p["dbg_q0"] = nc.declare_dram_parameter(
            "dbg_q0", [128, L], BF16, isOutput=True)
        p["dbg_k0"] = nc.declare_dram_parameter(
            "dbg_k0", [128, L], BF16, isOutput=True)
        p["dbg_vT0"] = nc.declare_dram_parameter(
            "dbg_vT0", [SCP, 128, 2, H, CH + 1], E4, isOutput=True)
        p["dbg_ab8"] = nc.declare_dram_parameter(
            "dbg_ab8", [KK, 128, 2, L], E4, isOutput=True)
        p["dbg_m1"] = nc.declare_dram_parameter(
            "dbg_m1", [128, 2, 512], F32, isOutput=True)
        p["dbg_ex8"] = nc.declare_dram_parameter(
            "dbg_ex8", [128, 2, 2, 512], E4, isOutput=True)
        p["dbg_ps2"] = nc.declare_dram_parameter(
            "dbg_ps2", [2, CH + 1, 512], F32, isOutput=True)
    p["x"] = nc.declare_dram_parameter("x", [C, L], F32, isOutput=False)
    p["q_w8"] = nc.declare_dram_parameter("q_w8", [128, PAIRS, KK, 2, 128],
                                          E4, isOutput=False)
    p["k_w8"] = nc.declare_dram_parameter("k_w8", [128, PAIRS, KK, 2, 128],
                                          E4, isOutput=False)
    p["v_w8"] = nc.declare_dram_parameter("v_w8", [128, LT, KK, 2, 512],
                                          E4, isOutput=False)
    p["p_w8"] = nc.declare_dram_parameter("p_w8", [128, NT, KK, 2, 128],
                                          E4, isOutput=False)
    p["q_b"] = nc.declare_dram_parameter("q_b", [128, PAIRS], F32, isOutput=False)
    p["k_b"] = nc.declare_dram_parameter("k_b", [128, PAIRS], F32, isOutput=False)
    p["proj_beff"] = nc.declare_dram_parameter("proj_beff", [128, NT], F32,
                                               isOutput=False)
    p["norm_w_c"] = nc.declare_dram_parameter("norm_w_c", [128, NT], F32,
                                              isOutput=False)
    p["norm_b_c"] = nc.declare_dram_parameter("norm_b_c", [128, NT], F32,
                                              isOutput=False)
    p["A_grp"] = nc.declare_dram_parameter("A_grp", [128, 4], F32,
                                           isOutput=False)
    p["A2T"] = nc.declare_dram_parameter("A2T", [4, 128], F32, isOutput=False)
    p["out"] = nc.declare_dram_parameter("out", [C, L], F32, isOutput=True)
    return p


def emit(nc, tc, ctx, params, out_handle=None):
    from contextlib import ExitStack

    x_d = params["x"]
    out_d = params["out"] if out_handle is None else out_handle
    x_ap, out_ap = x_d.ap(), out_d.ap()
    qw8, kw8 = params["q_w8"].ap(), params["k_w8"].ap()
    vw8, pw8 = params["v_w8"].ap(), params["p_w8"].ap()

    # ---- persistent pools --------------------------------------------
    consts = ctx.enter_context(tc.tile_pool(name="consts", bufs=1))
    xn_p = ctx.enter_context(tc.tile_pool(name="xn", bufs=NT))
    xn8_p = ctx.enter_context(tc.tile_pool(name="xn8", bufs=KK))
    vT_p = ctx.enter_context(tc.tile_pool(name="vT", bufs=SCP))
    ab8_p = ctx.enter_context(tc.tile_pool(name="ab8", bufs=LT * KK))
    # all 16 q/k tiles stay live through both tcn passes
    qk_p = ctx.enter_context(tc.tile_pool(name="qk", bufs=2 * PAIRS + 1))
    qkw_p = ctx.enter_context(tc.tile_pool(name="qkw", bufs=4))
    pw_p = ctx.enter_context(tc.tile_pool(name="pw", bufs=NT))
    # one shared 2-bank PSUM pool for the vT / qk-projection / proj
    # accumulators (never more than two of those streams active at once)
    aux_ps = ctx.enter_context(
        tc.tile_pool(name="auxps", bufs=1, space=bass.MemorySpace.PSUM))

    xn = []    # 8 x [128, L] f32: x_norm + proj_beff (residual + bias)
    xn8 = []   # 4 x [128, 2, L] e4m3: DoubleRow moving operand

    # ================= Phase 1: GroupNorm =============================
    with ExitStack() as ph1:
        xp = ph1.enter_context(tc.tile_pool(name="xp", bufs=NT // 2))
        scr_p = ph1.enter_context(tc.tile_pool(name="scr", bufs=2))
        gn_p = ph1.enter_context(tc.tile_pool(name="gn", bufs=1))
        gnps = ph1.enter_context(
            tc.tile_pool(name="gnps", bufs=2, space=bass.MemorySpace.PSUM))

        # x first: 4 wide DMAs (2 channel-tiles each) over 3 queues to
        # minimize per-descriptor round trips; consts/weights queue behind
        xt = []
        x_engs = [nc.sync, nc.gpsimd, nc.scalar]
        xr = x_ap.rearrange("(g p) l -> p g l", p=128)
        for d in range(4):
            big = xp.tile([128, 2, L], F32, tag="x_t", name="x_t")
            x_engs[d % 3].dma_start(out=big, in_=xr[:, 2 * d:2 * d + 2, :])
            xt.append(big[:, 0, :])
            xt.append(big[:, 1, :])

        def load_const(dram_name, shape, tag):
            t = consts.tile(shape, F32, tag=tag, name=tag)
            nc.scalar.dma_start(out=t, in_=params[dram_name].ap())
            return t

        ag_sb = load_const("A_grp", [128, 4], "ag")
        a2_sb = load_const("A2T", [4, 128], "a2")
        qb_sb = load_const("q_b", [128, PAIRS], "qb")
        kb_sb = load_const("k_b", [128, PAIRS], "kb")
        pb_sb = load_const("proj_beff", [128, NT], "pb")
        nw_sb = load_const("norm_w_c", [128, NT], "nw")
        nb_sb = load_const("norm_b_c", [128, NT], "nb")
        eps_sb = consts.tile([4, 1], F32, tag="eps", name="eps")
        nc.vector.memset(eps_sb, EPS)
        ebias_sb = consts.tile([128, 1], F32, tag="ebias", name="ebias")
        nc.vector.memset(ebias_sb, EXP_BIAS)
        ones_f = consts.tile([128, 2 * H], F32, tag="ones_f", name="ones_f")
        nc.vector.memset(ones_f, 1.0)

        # v weights (needed in phase 2), then proj weights (phase 4)
        vw_sb = []
        for n in range(LT):
            t = consts.tile([128, KK, 2, 512], E4, tag=f"vw{n}", name="vw_t")
            eng = nc.sync if n % 2 == 0 else nc.gpsimd
            eng.dma_start(out=t, in_=vw8[:, n])
            vw_sb.append(t)
        pw_sb = []
        for m in range(NT):
            t = pw_p.tile([128, KK, 2, 128], E4, tag="pw_t", name="pw_t")
            eng = nc.sync if m % 2 == 0 else nc.gpsimd
            eng.dma_start(out=t, in_=pw8[:, m])
            pw_sb.append(t)

        warm = gn_p.tile([4, 1], F32, tag="warm", name="warm")
        nc.scalar.activation(out=warm, in_=eps_sb,
                             func=mybir.ActivationFunctionType.Ln,
                             bias=eps_sb, scale=1.0)

        stats = gn_p.tile([128, 2 * NT], F32, tag="stats", name="stats")
        for t in range(NT):
            nc.vector.reduce_sum(
                out=stats[:, t:t + 1], in_=xt[t], axis=mybir.AxisListType.X)
            scr = scr_p.tile([128, L], F32, tag="scr", name="scr")
            nc.scalar.activation(
                out=scr, in_=xt[t],
                func=mybir.ActivationFunctionType.Square,
                accum_out=stats[:, NT + t:NT + t + 1])

        gps = gnps.tile([4, 2 * NT], F32, tag="gps", name="gps")
        nc.tensor.matmul(gps, ag_sb, stats)

        mv16 = gn_p.tile([4, 2 * NT], F32, tag="mv16", name="mv16")
        inv_n = 1.0 / (32 * L)
        nc.vector.tensor_scalar_mul(out=mv16[:, 0:NT], in0=gps[:, 0:NT],
                                    scalar1=inv_n)
        e2 = gn_p.tile([4, NT], F32, tag="e2", name="e2")
        nc.vector.tensor_scalar_mul(out=e2, in0=gps[:, NT:2 * NT],
                                    scalar1=inv_n)
        m2 = gn_p.tile([4, NT], F32, tag="m2", name="m2")
        nc.vector.tensor_tensor(out=m2, in0=mv16[:, 0:NT], in1=mv16[:, 0:NT],
                                op=mybir.AluOpType.mult)
        var = gn_p.tile([4, NT], F32, tag="var", name="var")
        nc.vector.tensor_tensor(out=var, in0=e2, in1=m2,
                                op=mybir.AluOpType.subtract)
        lnv = gn_p.tile([4, NT], F32, tag="lnv", name="lnv")
        nc.scalar.activation(out=lnv, in_=var,
                             func=mybir.ActivationFunctionType.Ln,
                             bias=eps_sb, scale=1.0)
        nc.scalar.activation(out=mv16[:, NT:2 * NT], in_=lnv,
                             func=mybir.ActivationFunctionType.Exp,
                             scale=-0.5)

        bc = gnps.tile([128, 2 * NT], F32, tag="bc", name="bc")
        nc.tensor.matmul(bc, a2_sb, mv16)

        scale_sb = gn_p.tile([128, NT], F32, tag="scale", name="scale")
        nc.vector.tensor_tensor(out=scale_sb, in0=nw_sb, in1=bc[:, NT:2 * NT],
                                op=mybir.AluOpType.mult)
        tmp = gn_p.tile([128, NT], F32, tag="tmp", name="tmp")
        nc.vector.tensor_tensor(out=tmp, in0=bc[:, 0:NT], in1=scale_sb,
                                op=mybir.AluOpType.mult)
        bias_sb = gn_p.tile([128, NT], F32, tag="bias", name="bias")
        nc.vector.tensor_tensor(out=bias_sb, in0=nb_sb, in1=tmp,
                                op=mybir.AluOpType.subtract)
        # residual copy also carries the proj bias: bias2 = bias + proj_beff
        bias2_sb = gn_p.tile([128, NT], F32, tag="bias2", name="bias2")
        nc.vector.tensor_tensor(out=bias2_sb, in0=bias_sb, in1=pb_sb,
                                op=mybir.AluOpType.add)

        for kk in range(KK):
            x8 = xn8_p.tile([128, 2, L], E4, tag="xn8_t", name="xn8_t")
            xn8.append(x8)
        for t in range(NT):
            nc.scalar.activation(
                out=xn8[t // 2][:, t % 2, :], in_=xt[t],
                func=mybir.ActivationFunctionType.Identity,
                scale=scale_sb[:, t:t + 1], bias=bias_sb[:, t:t + 1])
        for t in range(NT):
            xnt = xn_p.tile([128, L], F32, tag="xn_t", name="xn_t")
            nc.gpsimd.tensor_scalar(
                out=xnt, in0=xt[t],
                scalar1=scale_sb[:, t:t + 1], scalar2=bias2_sb[:, t:t + 1],
                op0=mybir.AluOpType.mult, op1=mybir.AluOpType.add)
            xn.append(xnt)

    # ================= Phase 2: v^T ===================================
    # vT8[scp][s, i, h, c] = v[64h+c, s-pos of chunk 2scp+i]; col 64 = 32.0
    qk_res = {}

    def qk_gen(j):
        """Emit pair j's q/k projection (DoubleRow) in small chunks."""
        wts = {}
        for name, w_ap in (("q", qw8), ("k", kw8)):
            wt = qkw_p.tile([128, KK, 2, 128], E4, tag="qkw_t", name="qkw_t")
            nc.sync.dma_start(out=wt, in_=w_ap[:, j])
            wts[name] = wt
        yield
        for name, b_sb in (("q", qb_sb), ("k", kb_sb)):
            wt = wts[name]
            dst = qk_p.tile([128, L], BF16, tag=f"{name}_j", name=f"{name}_j")
            for n in range(LT):
                acc = aux_ps.tile([128, 512], F32, tag="aux", name="aux")
                for kk in range(KK):
                    nc.tensor.matmul(
                        acc, wt[:, kk], xn8[kk][:, :, n * 512:(n + 1) * 512],
                        perf_mode=DR, start=(kk == 0), stop=(kk == KK - 1))
                    if kk == 1:
                        yield
                nc.vector.tensor_scalar_add(
                    out=dst[:, n * 512:(n + 1) * 512], in0=acc,
                    scalar1=b_sb[:, j:j + 1])
                yield
            qk_res.setdefault(j, {})[name] = dst

    # ================= Phase 2: v^T ===================================
    vT8 = []
    with ExitStack() as ph2:
        vps = ph2.enter_context(
            tc.tile_pool(name="vps", bufs=4, space=bass.MemorySpace.PSUM))
        for scp in range(SCP):
            vt = vT_p.tile([128, 2, H, CH + 1], E4, tag="vT_t", name="vT_t")
            nc.vector.tensor_copy(
                out=vt[:, :, :, CH],
                in_=ones_f.rearrange("p (i h) -> p i h", i=2))
            vT8.append(vt)
        for n in range(LT):
            for g in range(2):
                accs = [vps.tile([128, 512], F32, tag="vac", name="vac")
                        for _ in range(4)]
                for kk in range(KK):
                    for li, lc in enumerate(range(g * 4, g * 4 + 4)):
                        nc.tensor.matmul(
                            accs[li],
                            xn8[kk][:, :, lc * 128:(lc + 1) * 128],
                            vw_sb[n][:, kk],
                            perf_mode=DR, start=(kk == 0), stop=(kk == KK - 1))
                for li, lc in enumerate(range(g * 4, g * 4 + 4)):
                    nc.vector.tensor_scalar_mul(
                        out=vT8[lc // 2][:, lc % 2, n * 8:(n + 1) * 8, 0:CH],
                        in0=accs[li].rearrange("p (h c) -> p h c", c=CH),
                        scalar1=1.0 / WS)

    # ============ Phase 3: attention (tcn-outer) ======================
    # tcn0 interleaves the next pair's q/k projection; tcn1 interleaves
    # the proj matmuls for the already-complete tcn0 half of ab8, so the
    # epilogue tail is only the tcn1 half of proj.
    ab8 = [[ab8_p.tile([128, 2, 512], E4, tag="ab8_t", name="ab8_t")
            for _ in range(KK)] for _ in range(LT)]

    def proj_emit(n, out_p, use_m1=False):
        for m in range(NT):
            if use_m1:
                mt = m1_p.tile([128, 2, 512], F32, tag="m1", name="m1")
                acc = mt[:, 0, :]
            else:
                acc = aux_ps.tile([128, 512], F32, tag="aux", name="aux")
            for kk in range(KK):
                nc.tensor.matmul(
                    acc, pw_sb[m][:, kk],
                    ab8[n][kk],
                    perf_mode=DR, start=(kk == 0), stop=(kk == KK - 1))
                if kk == 1:
                    yield
            o_sb = out_p.tile([128, 512], F32, tag="o_sb", name="o_sb")
            nc.vector.scalar_tensor_tensor(
                out=o_sb, in0=acc, scalar=1.0 / WS,
                in1=xn[m][:, n * 512:(n + 1) * 512],
                op0=mybir.AluOpType.mult, op1=mybir.AluOpType.add)
            eng = nc.sync if m % 2 == 0 else nc.gpsimd
            eng.dma_start(
                out=out_ap[m * 128:(m + 1) * 128, n * 512:(n + 1) * 512],
                in_=o_sb)
            yield

    with ExitStack() as ph3:
        ex_p = ph3.enter_context(tc.tile_pool(name="exp", bufs=4))
        rc_p = ph3.enter_context(tc.tile_pool(name="rcp", bufs=4))
        out_p = ph3.enter_context(tc.tile_pool(name="outp", bufs=4))
        m1_p = ph3.enter_context(
            tc.tile_pool(name="m1p", bufs=2, space=bass.MemorySpace.PSUM))
        ps2_p = ph3.enter_context(
            tc.tile_pool(name="ps2p", bufs=3, space=bass.MemorySpace.PSUM))

        for _ in qk_gen(0):
            pass

        proj0 = None
        for tcn in range(LT):
            if tcn == 1:
                proj0 = proj_emit(0, out_p)
            for j in range(PAIRS):
                if tcn == 0:
                    nxt = qk_gen(j + 1) if j + 1 < PAIRS else None
                else:
                    nxt = proj0
                q_j = qk_res[j]["q"]
                k_j = qk_res[j]["k"]

                ps2 = [ps2_p.tile([CH + 1, 512], F32, tag="ps2", name="ps2")
                       for _ in range(2)]
                for scp in range(SCP):
                    ex8 = ex_p.tile([128, 2, 2, 512], E4, tag="ex8",
                                    name="ex8")
                    for i in range(2):
                        sc = 2 * scp + i
                        m1 = m1_p.tile([128, 2, 512], F32, tag="m1",
                                       name="m1")
                        for par in range(2):
                            base = CH * par
                            nc.tensor.matmul(
                                m1[:, par, :],
                                k_j[base:base + CH,
                                    sc * 128:(sc + 1) * 128],
                                q_j[base:base + CH,
                                    tcn * 512:(tcn + 1) * 512])
                        nc.scalar.activation(
                            out=ex8[:, i, :, :], in_=m1,
                            func=mybir.ActivationFunctionType.Exp,
                            scale=0.125 / (WS * WS), bias=ebias_sb)
                    for par in range(2):
                        nc.tensor.matmul(
                            ps2[par], vT8[scp][:, :, 2 * j + par, :],
                            ex8[:, :, par, :],
                            perf_mode=DR,
                            start=(scp == 0), stop=(scp == SCP - 1))
                    if nxt is not None:
                        next(nxt, None)
                        if tcn == 1:
                            next(nxt, None)
                # normalize: a = a_raw * (1 / S); S sits in psum row 64.
                # Order matters on HW: the partition_broadcast (gpsimd)
                # input must come from a plain DVE copy, and the custom-DVE
                # reciprocal output must be consumed by DVE only.
                for par in range(2):
                    base = CH * par
                    s_sb = rc_p.tile([1, 512], F32, tag="s_sb", name="s_sb")
                    nc.vector.tensor_copy(out=s_sb,
                                          in_=ps2[par][CH:CH + 1, :])
                    sbb = rc_p.tile([CH, 512], F32, tag="sbb", name="sbb")
                    nc.gpsimd.partition_broadcast(sbb, s_sb, channels=CH)
                    rc64 = rc_p.tile([CH, 512], F32, tag="rc64", name="rc64")
                    nc.vector.reciprocal_approx_fast(out=rc64, in_=sbb)
                    nc.vector.tensor_tensor(
                        out=ab8[tcn][j // 2][base:base + CH, j % 2, :],
                        in0=ps2[par][0:CH, :], in1=rc64,
                        op=mybir.AluOpType.mult)
                if tcn == 0 and nxt is not None:
                    for _ in nxt:
                        pass
        # ======== proj tail: drain n=0 remainder, then n=1 ============
        if proj0 is not None:
            for _ in proj0:
                pass
        for _ in proj_emit(1, out_p, use_m1=True):
            pass


_CACHED = {}


def build_program(repeats=1):
    key = ("nc", repeats)
    if key in _CACHED:
        return _CACHED[key]
    from contextlib import ExitStack

    nc = bacc.Bacc("TRN2", target_bir_lowering=False, debug=False)
    with tile.TileContext(nc) as tc:
        params = declare_params(nc)
        for rep in range(repeats):
            out_h = None
            if rep > 0:
                out_h = nc.dram_tensor(f"out_scratch{rep}", [C, L], F32)
            with ExitStack() as ctx:
                emit(nc, tc, ctx, params, out_h)
    nc.compile()
    _CACHED[key] = nc
    return nc


def to_e4(a):
    return np.clip(np.asarray(a, np.float32), -240, 240).astype(
        ml_dtypes.float8_e4m3)


def host_pack(norm_w, norm_b, qkv_w, qkv_b, proj_w, proj_b):
    """Precompute packed weight layouts (all plain numpy)."""
    f = np.float32
    qkv_w = np.asarray(qkv_w, f)
    qkv_b = np.asarray(qkv_b, f)
    proj_w = np.asarray(proj_w, f)
    proj_b = np.asarray(proj_b, f)

    idx_q = np.empty(C, np.int64)
    idx_k = np.empty(C, np.int64)
    for j in range(PAIRS):
        for p in range(128):
            h = 2 * j + p // CH
            i = p % CH
            idx_q[j * 128 + p] = 192 * h + i
            idx_k[j * 128 + p] = 192 * h + CH + i
    idx_v = np.empty(C, np.int64)
    for h in range(H):
        idx_v[CH * h:CH * (h + 1)] = 192 * h + 2 * CH + np.arange(CH)

    def pack_w(w, out_grp, out_sz):
        # w: [1024 out-rows, 1024 in-ch] -> [128, out_grp, KK, 2, out_sz]
        a = (WS * w).reshape(out_grp, out_sz, KK, 2, 128)
        return to_e4(np.ascontiguousarray(a.transpose(4, 0, 2, 3, 1)))

    q_w8 = pack_w(qkv_w[idx_q, :], PAIRS, 128)
    k_w8 = pack_w(qkv_w[idx_k, :], PAIRS, 128)
    v_w8 = pack_w(qkv_w[idx_v, :], LT, 512)
    p_w8 = pack_w(proj_w, NT, 128)

    # q/k are kept x32-scaled (psum + 32*bias); exp scale divides by 32^2
    q_b = np.ascontiguousarray(WS * qkv_b[idx_q].reshape(PAIRS, 128).T)
    k_b = np.ascontiguousarray(WS * qkv_b[idx_k].reshape(PAIRS, 128).T)
    pbe = proj_b + proj_w @ qkv_b[idx_v]
    proj_beff = np.ascontiguousarray(pbe.astype(f).reshape(NT, 128).T)

    norm_w_c = np.ascontiguousarray(np.asarray(norm_w, f).reshape(NT, 128).T)
    norm_b_c = np.ascontiguousarray(np.asarray(norm_b, f).reshape(NT, 128).T)

    pp = np.arange(128)
    A_grp = (pp[:, None] // 32 == np.arange(4)[None, :]).astype(f)
    A2T = np.ascontiguousarray(A_grp.T)

    return dict(
        q_w8=q_w8, k_w8=k_w8, v_w8=v_w8, p_w8=p_w8,
        q_b=q_b, k_b=k_b, proj_beff=proj_beff,
        norm_w_c=norm_w_c, norm_b_c=norm_b_c, A_grp=A_grp, A2T=A2T,
    )


def kernel(x, norm_w, norm_b, qkv_w, qkv_b, proj_w, proj_b, _trace=False):
    x = np.asarray(x, np.float32)
    shared = host_pack(norm_w, norm_b, qkv_w, qkv_b, proj_w, proj_b)
    nc = build_program()
    in_maps = [dict(shared, x=np.ascontiguousarray(x[i])) for i in range(B)]
    res = run_bass_kernel_spmd(nc, in_maps, list(range(B)), trace=_trace)
    out = np.stack([res.results[i]["out"] for i in range(B)], axis=0)
    if _trace:
        kernel._last_results = res
    return out.astype(np.float32)


# revision 35
# speedup vs baseline: 1.3041x; 1.0284x over previous
"""Trainium2 Bass kernel for nn_AttentionBlock (B=8, C=1024, L=1024, H=16, G=32).

Data-parallel over batch: one sample per NeuronCore, no collectives.
fp8e4 (e4m3, TRN max 240) DoubleRow matmuls carry the heavy GEMMs: each
DoubleRow instruction contracts 256 rows (two 128-deep k-subtiles packed
into the PE array) per moving column, halving PE time vs bf16.

Per core:
  1. GroupNorm(32): per-channel sum (DVE) + sumsq (ACT Square-accum),
     cross-partition group reduce/broadcast via tiny fp32 matmuls,
     rsqrt via Ln/Exp (single ACT table set).  Apply produces
       - xn8: fp8 x_norm in DoubleRow pair-interleaved layout
         [128, 2, L] x 4 tiles (moving operand for qkv projections),
       - xn':  fp32 x_norm + proj_bias_eff (residual + epilogue bias,
         computed on the Pool engine to keep DVE free).
     Weights are host-scaled x32 (power of two) and clipped to +-240.
  2. v^T via operand swap (stationary = xn8 chunk), DoubleRow, directly
     in [128, 2(s-parity), 16 heads, 65] fp8 tiles; column 64 holds the
     constant 32 so the attention mm2 emits the softmax denominator.
  3. Attention, ACT-bound: per s-chunk one [128, 2(head), 512] PSUM score
     tile, one 1024-wide ACT exp (scale=1/8, bias=-2) writing fp8 into
     the DoubleRow-interleaved ex8 tile, then one DoubleRow mm2 per head
     per s-chunk-pair.  Softmax needs no max subtraction (|z|/8 <= ~6.1;
     bias -2 keeps exp <= ~60 < 240).  Normalize: DVE reciprocal of the
     denominator row straight out of PSUM, Pool-engine partition
     broadcast, one DVE multiply into the fp8 proj operand.  The next
     pair's q/k projection (DoubleRow) is interleaved one chunk per
     s-step so the PE never drains at pair boundaries.
  4. proj (DoubleRow) + (xn + bias_eff) residual epilogue, DMA out on
     two queues.  v-bias is folded into proj bias on the host.
"""

import numpy as np
import ml_dtypes

import concourse.bass as bass
import concourse.bacc as bacc
import concourse.tile as tile
from concourse import mybir
from concourse.bass_utils import run_bass_kernel_spmd

F32 = mybir.dt.float32
BF16 = mybir.dt.bfloat16
E4 = mybir.dt.float8e4
DR = mybir.MatmulPerfMode.DoubleRow

B, C, L, H = 8, 1024, 1024, 16
GROUPS = 32
CH = C // H          # 64 per-head channels
EPS = 1e-5
NT = C // 128        # 8 channel tiles
LT = L // 512        # 2 free-dim chunks of 512
PAIRS = H // 2       # 8 head pairs
KK = 4               # DoubleRow contraction steps (4 x 256 = 1024)
SCP = 4              # s-chunk pairs per 1024 positions
WS = 32.0            # host weight scale (power of 2)
EXP_BIAS = -2.0      # exp(z/8 + EXP_BIAS): max ~exp(4.1)=60 << 240


DEBUG_DUMPS = False


def declare_params(nc):
    p = {}
    if DEBUG_DUMPS:
        p["dbg_xn8"] = nc.declare_dram_parameter(
            "dbg_xn8", [KK, 128, 2, L], E4, isOutput=True)
        p["dbg_q0"] = nc.declare_dram_parameter(
            "dbg_q0", [128, L], BF16, isOutput=True)
        p["dbg_k0"] = nc.declare_dram_parameter(
            "dbg_k0", [128, L], BF16, isOutput=True)
        p["dbg_vT0"] = nc.declare_dram_parameter(
            "dbg_vT0", [SCP, 128, 2, H, CH + 1], E4, isOutput=True)
        p["dbg_ab8"] = nc.declare_dram_parameter(
            "dbg_ab8", [KK, 128, 2, L], E4, isOutput=True)
        p["dbg_m1"] = nc.declare_dram_parameter(
            "dbg_m1", [128, 2, 512], F32, isOutput=True)
        p["dbg_ex8"] = nc.declare_dram_parameter(
            "dbg_ex8", [128, 2, 2, 512], E4, isOutput=True)
        p["dbg_ps2"] = nc.declare_dram_parameter(
            "dbg_ps2", [2, CH + 1, 512], F32, isOutput=True)
    p["x"] = nc.declare_dram_parameter("x", [C, L], F32, isOutput=False)
    p["q_w8"] = nc.declare_dram_parameter("q_w8", [128, PAIRS, KK, 2, 128],
                                          E4, isOutput=False)
    p["k_w8"] = nc.declare_dram_parameter("k_w8", [128, PAIRS, KK, 2, 128],
                                          E4, isOutput=False)
    p["v_w8"] = nc.declare_dram_parameter("v_w8", [128, LT, KK, 2, 512],
                                          E4, isOutput=False)
    p["p_w8"] = nc.declare_dram_parameter("p_w8", [128, NT, KK, 2, 128],
                                          E4, isOutput=False)
    p["q_b"] = nc.declare_dram_parameter("q_b", [128, PAIRS], F32, isOutput=False)
    p["k_b"] = nc.declare_dram_parameter("k_b", [128, PAIRS], F32, isOutput=False)
    p["proj_beff"] = nc.declare_dram_parameter("proj_beff", [128, NT], F32,
                                               isOutput=False)
    p["norm_w_c"] = nc.declare_dram_parameter("norm_w_c", [128, NT], F32,
                                              isOutput=False)
    p["norm_b_c"] = nc.declare_dram_parameter("norm_b_c", [128, NT], F32,
                                              isOutput=False)
    p["A_grp"] = nc.declare_dram_parameter("A_grp", [128, 4], F32,
                                           isOutput=False)
    p["A2T"] = nc.declare_dram_parameter("A2T", [4, 128], F32, isOutput=False)
    p["out"] = nc.declare_dram_parameter("out", [C, L], F32, isOutput=True)
    return p


def emit(nc, tc, ctx, params, out_handle=None):
    from contextlib import ExitStack

    x_d = params["x"]
    out_d = params["out"] if out_handle is None else out_handle
    x_ap, out_ap = x_d.ap(), out_d.ap()
    qw8, kw8 = params["q_w8"].ap(), params["k_w8"].ap()
    vw8, pw8 = params["v_w8"].ap(), params["p_w8"].ap()

    # ---- persistent pools --------------------------------------------
    consts = ctx.enter_context(tc.tile_pool(name="consts", bufs=1))
    xn_p = ctx.enter_context(tc.tile_pool(name="xn", bufs=NT))
    xn8_p = ctx.enter_context(tc.tile_pool(name="xn8", bufs=KK))
    vT_p = ctx.enter_context(tc.tile_pool(name="vT", bufs=SCP))
    ab8_p = ctx.enter_context(tc.tile_pool(name="ab8", bufs=LT * KK))
    # all 16 q/k tiles stay live through both tcn passes
    qk_p = ctx.enter_context(tc.tile_pool(name="qk", bufs=2 * PAIRS + 1))
    qkw_p = ctx.enter_context(tc.tile_pool(name="qkw", bufs=4))
    pw_p = ctx.enter_context(tc.tile_pool(name="pw", bufs=NT))
    # one shared 2-bank PSUM pool for the vT / qk-projection / proj
    # accumulators (never more than two of those streams active at once)
    aux_ps = ctx.enter_context(
        tc.tile_pool(name="auxps", bufs=1, space=bass.MemorySpace.PSUM))

    xn = []    # 8 x [128, L] f32: x_norm + proj_beff (residual + bias)
    xn8 = []   # 4 x [128, 2, L] e4m3: DoubleRow moving operand

    # ================= Phase 1: GroupNorm =============================
    with ExitStack() as ph1:
        xp = ph1.enter_context(tc.tile_pool(name="xp", bufs=NT // 2))
        scr_p = ph1.enter_context(tc.tile_pool(name="scr", bufs=2))
        gn_p = ph1.enter_context(tc.tile_pool(name="gn", bufs=1))
        gnps = ph1.enter_context(
            tc.tile_pool(name="gnps", bufs=2, space=bass.MemorySpace.PSUM))

        # x first: 4 wide DMAs (2 channel-tiles each) over 3 queues to
        # minimize per-descriptor round trips; consts/weights queue behind
        xt = []
        x_engs = [nc.sync, nc.gpsimd, nc.scalar]
        xr = x_ap.rearrange("(g p) l -> p g l", p=128)
        for d in range(4):
            big = xp.tile([128, 2, L], F32, tag="x_t", name="x_t")
            x_engs[d % 3].dma_start(out=big, in_=xr[:, 2 * d:2 * d + 2, :])
            xt.append(big[:, 0, :])
            xt.append(big[:, 1, :])

        def load_const(dram_name, shape, tag):
            t = consts.tile(shape, F32, tag=tag, name=tag)
            nc.scalar.dma_start(out=t, in_=params[dram_name].ap())
            return t

        ag_sb = load_const("A_grp", [128, 4], "ag")
        a2_sb = load_const("A2T", [4, 128], "a2")
        qb_sb = load_const("q_b", [128, PAIRS], "qb")
        kb_sb = load_const("k_b", [128, PAIRS], "kb")
        pb_sb = load_const("proj_beff", [128, NT], "pb")
        nw_sb = load_const("norm_w_c", [128, NT], "nw")
        nb_sb = load_const("norm_b_c", [128, NT], "nb")
        eps_sb = consts.tile([4, 1], F32, tag="eps", name="eps")
        nc.vector.memset(eps_sb, EPS)
        ebias_sb = consts.tile([128, 1], F32, tag="ebias", name="ebias")
        nc.vector.memset(ebias_sb, EXP_BIAS)
        ones_f = consts.tile([128, 2 * H], F32, tag="ones_f", name="ones_f")
        nc.vector.memset(ones_f, 1.0)

        # v weights (needed in phase 2), then proj weights (phase 4)
        vw_sb = []
        for n in range(LT):
            t = consts.tile([128, KK, 2, 512], E4, tag=f"vw{n}", name="vw_t")
            eng = nc.sync if n % 2 == 0 else nc.gpsimd
            eng.dma_start(out=t, in_=vw8[:, n])
            vw_sb.append(t)
        pw_sb = []
        for m in range(NT):
            t = pw_p.tile([128, KK, 2, 128], E4, tag="pw_t", name="pw_t")
            eng = nc.sync if m % 2 == 0 else nc.gpsimd
            eng.dma_start(out=t, in_=pw8[:, m])
            pw_sb.append(t)

        warm = gn_p.tile([4, 1], F32, tag="warm", name="warm")
        nc.scalar.activation(out=warm, in_=eps_sb,
                             func=mybir.ActivationFunctionType.Ln,
                             bias=eps_sb, scale=1.0)

        stats = gn_p.tile([128, 2 * NT], F32, tag="stats", name="stats")
        for t in range(NT):
            nc.vector.reduce_sum(
                out=stats[:, t:t + 1], in_=xt[t], axis=mybir.AxisListType.X)
            scr = scr_p.tile([128, L], F32, tag="scr", name="scr")
            nc.scalar.activation(
                out=scr, in_=xt[t],
                func=mybir.ActivationFunctionType.Square,
                accum_out=stats[:, NT + t:NT + t + 1])

        gps = gnps.tile([4, 2 * NT], F32, tag="gps", name="gps")
        nc.tensor.matmul(gps, ag_sb, stats)

        mv16 = gn_p.tile([4, 2 * NT], F32, tag="mv16", name="mv16")
        inv_n = 1.0 / (32 * L)
        nc.vector.tensor_scalar_mul(out=mv16[:, 0:NT], in0=gps[:, 0:NT],
                                    scalar1=inv_n)
        e2 = gn_p.tile([4, NT], F32, tag="e2", name="e2")
        nc.vector.tensor_scalar_mul(out=e2, in0=gps[:, NT:2 * NT],
                                    scalar1=inv_n)
        m2 = gn_p.tile([4, NT], F32, tag="m2", name="m2")
        nc.vector.tensor_tensor(out=m2, in0=mv16[:, 0:NT], in1=mv16[:, 0:NT],
                                op=mybir.AluOpType.mult)
        var = gn_p.tile([4, NT], F32, tag="var", name="var")
        nc.vector.tensor_tensor(out=var, in0=e2, in1=m2,
                                op=mybir.AluOpType.subtract)
        lnv = gn_p.tile([4, NT], F32, tag="lnv", name="lnv")
        nc.scalar.activation(out=lnv, in_=var,
                             func=mybir.ActivationFunctionType.Ln,
                             bias=eps_sb, scale=1.0)
        nc.scalar.activation(out=mv16[:, NT:2 * NT], in_=lnv,
                             func=mybir.ActivationFunctionType.Exp,
                             scale=-0.5)

        bc = gnps.tile([128, 2 * NT], F32, tag="bc", name="bc")
        nc.tensor.matmul(bc, a2_sb, mv16)

        scale_sb = gn_p.tile([128, NT], F32, tag="scale", name="scale")
        nc.vector.tensor_tensor(out=scale_sb, in0=nw_sb, in1=bc[:, NT:2 * NT],
                                op=mybir.AluOpType.mult)
        tmp = gn_p.tile([128, NT], F32, tag="tmp", name="tmp")
        nc.vector.tensor_tensor(out=tmp, in0=bc[:, 0:NT], in1=scale_sb,
                                op=mybir.AluOpType.mult)
        bias_sb = gn_p.tile([128, NT], F32, tag="bias", name="bias")
        nc.vector.tensor_tensor(out=bias_sb, in0=nb_sb, in1=tmp,
                                op=mybir.AluOpType.subtract)
        # residual copy also carries the proj bias: bias2 = bias + proj_beff
        bias2_sb = gn_p.tile([128, NT], F32, tag="bias2", name="bias2")
        nc.vector.tensor_tensor(out=bias2_sb, in0=bias_sb, in1=pb_sb,
                                op=mybir.AluOpType.add)

        for kk in range(KK):
            x8 = xn8_p.tile([128, 2, L], E4, tag="xn8_t", name="xn8_t")
            xn8.append(x8)
        for t in range(NT):
            nc.scalar.activation(
                out=xn8[t // 2][:, t % 2, :], in_=xt[t],
                func=mybir.ActivationFunctionType.Identity,
                scale=scale_sb[:, t:t + 1], bias=bias_sb[:, t:t + 1])
        for t in range(NT):
            xnt = xn_p.tile([128, L], F32, tag="xn_t", name="xn_t")
            nc.gpsimd.tensor_scalar(
                out=xnt, in0=xt[t],
                scalar1=scale_sb[:, t:t + 1], scalar2=bias2_sb[:, t:t + 1],
                op0=mybir.AluOpType.mult, op1=mybir.AluOpType.add)
            xn.append(xnt)
        # dummy broadcast: forces the gpsimd DSP library swap (~7us) to
        # happen here, overlapped with vT/qk, not inside the attention loop
        warmb = gn_p.tile([2, 16], F32, tag="warmb", name="warmb")
        nc.gpsimd.partition_broadcast(warmb, ones_f[0:1, 0:16], channels=2)

    # ================= Phase 2: v^T ===================================
    # vT8[scp][s, i, h, c] = v[64h+c, s-pos of chunk 2scp+i]; col 64 = 32.0
    qk_res = {}

    def qk_gen(j):
        """Emit pair j's q/k projection (DoubleRow) in small chunks."""
        wts = {}
        for name, w_ap in (("q", qw8), ("k", kw8)):
            wt = qkw_p.tile([128, KK, 2, 128], E4, tag="qkw_t", name="qkw_t")
            nc.sync.dma_start(out=wt, in_=w_ap[:, j])
            wts[name] = wt
        yield
        for name, b_sb in (("q", qb_sb), ("k", kb_sb)):
            wt = wts[name]
            dst = qk_p.tile([128, L], BF16, tag=f"{name}_j", name=f"{name}_j")
            for n in range(LT):
                acc = aux_ps.tile([128, 512], F32, tag="aux", name="aux")
                for kk in range(KK):
                    nc.tensor.matmul(
                        acc, wt[:, kk], xn8[kk][:, :, n * 512:(n + 1) * 512],
                        perf_mode=DR, start=(kk == 0), stop=(kk == KK - 1))
                    if kk == 1:
                        yield
                nc.vector.tensor_scalar_add(
                    out=dst[:, n * 512:(n + 1) * 512], in0=acc,
                    scalar1=b_sb[:, j:j + 1])
                yield
            qk_res.setdefault(j, {})[name] = dst

    # ================= Phase 2: v^T ===================================
    vT8 = []
    with ExitStack() as ph2:
        vps = ph2.enter_context(
            tc.tile_pool(name="vps", bufs=4, space=bass.MemorySpace.PSUM))
        for scp in range(SCP):
            vt = vT_p.tile([128, 2, H, CH + 1], E4, tag="vT_t", name="vT_t")
            nc.vector.tensor_copy(
                out=vt[:, :, :, CH],
                in_=ones_f.rearrange("p (i h) -> p i h", i=2))
            vT8.append(vt)
        for n in range(LT):
            for g in range(2):
                accs = [vps.tile([128, 512], F32, tag="vac", name="vac")
                        for _ in range(4)]
                for kk in range(KK):
                    for li, lc in enumerate(range(g * 4, g * 4 + 4)):
                        nc.tensor.matmul(
                            accs[li],
                            xn8[kk][:, :, lc * 128:(lc + 1) * 128],
                            vw_sb[n][:, kk],
                            perf_mode=DR, start=(kk == 0), stop=(kk == KK - 1))
                for li, lc in enumerate(range(g * 4, g * 4 + 4)):
                    nc.vector.tensor_scalar_mul(
                        out=vT8[lc // 2][:, lc % 2, n * 8:(n + 1) * 8, 0:CH],
                        in0=accs[li].rearrange("p (h c) -> p h c", c=CH),
                        scalar1=1.0 / WS)

    # ============ Phase 3: attention (tcn-outer) ======================
    # tcn0 interleaves the next pair's q/k projection; tcn1 interleaves
    # the proj matmuls for the already-complete tcn0 half of ab8, so the
    # epilogue tail is only the tcn1 half of proj.
    ab8 = [[ab8_p.tile([128, 2, 512], E4, tag="ab8_t", name="ab8_t")
            for _ in range(KK)] for _ in range(LT)]

    def proj_emit(n, out_p, use_m1=False):
        for m in range(NT):
            if use_m1:
                mt = m1_p.tile([128, 2, 512], F32, tag="m1", name="m1")
                acc = mt[:, 0, :]
            else:
                acc = aux_ps.tile([128, 512], F32, tag="aux", name="aux")
            for kk in range(KK):
                nc.tensor.matmul(
                    acc, pw_sb[m][:, kk],
                    ab8[n][kk],
                    perf_mode=DR, start=(kk == 0), stop=(kk == KK - 1))
                if kk == 1:
                    yield
            o_sb = out_p.tile([128, 512], F32, tag="o_sb", name="o_sb")
            nc.vector.scalar_tensor_tensor(
                out=o_sb, in0=acc, scalar=1.0 / WS,
                in1=xn[m][:, n * 512:(n + 1) * 512],
                op0=mybir.AluOpType.mult, op1=mybir.AluOpType.add)
            eng = nc.sync if m % 2 == 0 else nc.gpsimd
            eng.dma_start(
                out=out_ap[m * 128:(m + 1) * 128, n * 512:(n + 1) * 512],
                in_=o_sb)
            yield

    with ExitStack() as ph3:
        ex_p = ph3.enter_context(tc.tile_pool(name="exp", bufs=4))
        rc_p = ph3.enter_context(tc.tile_pool(name="rcp", bufs=4))
        out_p = ph3.enter_context(tc.tile_pool(name="outp", bufs=4))
        m1_p = ph3.enter_context(
            tc.tile_pool(name="m1p", bufs=2, space=bass.MemorySpace.PSUM))
        ps2_p = ph3.enter_context(
            tc.tile_pool(name="ps2p", bufs=3, space=bass.MemorySpace.PSUM))

        for _ in qk_gen(0):
            pass

        proj0 = None
        for tcn in range(LT):
            if tcn == 1:
                proj0 = proj_emit(0, out_p)
            for j in range(PAIRS):
                if tcn == 0:
                    nxt = qk_gen(j + 1) if j + 1 < PAIRS else None
                else:
                    nxt = proj0
                q_j = qk_res[j]["q"]
                k_j = qk_res[j]["k"]

                ps2 = [ps2_p.tile([CH + 1, 512], F32, tag="ps2", name="ps2")
                       for _ in range(2)]
                for scp in range(SCP):
                    ex8 = ex_p.tile([128, 2, 2, 512], E4, tag="ex8",
                                    name="ex8")
                    for i in range(2):
                        sc = 2 * scp + i
                        m1 = m1_p.tile([128, 2, 512], F32, tag="m1",
                                       name="m1")
                        for par in range(2):
                            base = CH * par
                            nc.tensor.matmul(
                                m1[:, par, :],
                                k_j[base:base + CH,
                                    sc * 128:(sc + 1) * 128],
                                q_j[base:base + CH,
                                    tcn * 512:(tcn + 1) * 512])
                        nc.scalar.activation(
                            out=ex8[:, i, :, :], in_=m1,
                            func=mybir.ActivationFunctionType.Exp,
                            scale=0.125 / (WS * WS), bias=ebias_sb)
                    for par in range(2):
                        nc.tensor.matmul(
                            ps2[par], vT8[scp][:, :, 2 * j + par, :],
                            ex8[:, :, par, :],
                            perf_mode=DR,
                            start=(scp == 0), stop=(scp == SCP - 1))
                    if nxt is not None:
                        next(nxt, None)
                        if tcn == 1:
                            next(nxt, None)
                # normalize: a = a_raw * (1 / S); S sits in psum row 64.
                # Order matters on HW: the partition_broadcast (gpsimd)
                # input must come from a plain DVE copy, and the custom-DVE
                # reciprocal output must be consumed by DVE only.
                for par in range(2):
                    base = CH * par
                    s_sb = rc_p.tile([1, 512], F32, tag="s_sb", name="s_sb")
                    nc.vector.tensor_copy(out=s_sb,
                                          in_=ps2[par][CH:CH + 1, :])
                    sbb = rc_p.tile([CH, 512], F32, tag="sbb", name="sbb")
                    nc.gpsimd.partition_broadcast(sbb, s_sb, channels=CH)
                    rc64 = rc_p.tile([CH, 512], F32, tag="rc64", name="rc64")
                    nc.vector.reciprocal_approx_fast(out=rc64, in_=sbb)
                    nc.vector.tensor_tensor(
                        out=ab8[tcn][j // 2][base:base + CH, j % 2, :],
                        in0=ps2[par][0:CH, :], in1=rc64,
                        op=mybir.AluOpType.mult)
                if tcn == 0 and nxt is not None:
                    for _ in nxt:
                        pass
        # ======== proj tail: drain n=0 remainder, then n=1 ============
        if proj0 is not None:
            for _ in proj0:
                pass
        for _ in proj_emit(1, out_p, use_m1=True):
            pass


_CACHED = {}


def build_program(repeats=1):
    key = ("nc", repeats)
    if key in _CACHED:
        return _CACHED[key]
    from contextlib import ExitStack

    nc = bacc.Bacc("TRN2", target_bir_lowering=False, debug=False)
    with tile.TileContext(nc) as tc:
        params = declare_params(nc)
        for rep in range(repeats):
            out_h = None
            if rep > 0:
                out_h = nc.dram_tensor(f"out_scratch{rep}", [C, L], F32)
            with ExitStack() as ctx:
                emit(nc, tc, ctx, params, out_h)
    nc.compile()
    _CACHED[key] = nc
    return nc


def to_e4(a):
    return np.clip(np.asarray(a, np.float32), -240, 240).astype(
        ml_dtypes.float8_e4m3)


def host_pack(norm_w, norm_b, qkv_w, qkv_b, proj_w, proj_b):
    """Precompute packed weight layouts (all plain numpy)."""
    f = np.float32
    qkv_w = np.asarray(qkv_w, f)
    qkv_b = np.asarray(qkv_b, f)
    proj_w = np.asarray(proj_w, f)
    proj_b = np.asarray(proj_b, f)

    idx_q = np.empty(C, np.int64)
    idx_k = np.empty(C, np.int64)
    for j in range(PAIRS):
        for p in range(128):
            h = 2 * j + p // CH
            i = p % CH
            idx_q[j * 128 + p] = 192 * h + i
            idx_k[j * 128 + p] = 192 * h + CH + i
    idx_v = np.empty(C, np.int64)
    for h in range(H):
        idx_v[CH * h:CH * (h + 1)] = 192 * h + 2 * CH + np.arange(CH)

    def pack_w(w, out_grp, out_sz):
        # w: [1024 out-rows, 1024 in-ch] -> [128, out_grp, KK, 2, out_sz]
        a = (WS * w).reshape(out_grp, out_sz, KK, 2, 128)
        return to_e4(np.ascontiguousarray(a.transpose(4, 0, 2, 3, 1)))

    q_w8 = pack_w(qkv_w[idx_q, :], PAIRS, 128)
    k_w8 = pack_w(qkv_w[idx_k, :], PAIRS, 128)
    v_w8 = pack_w(qkv_w[idx_v, :], LT, 512)
    p_w8 = pack_w(proj_w, NT, 128)

    # q/k are kept x32-scaled (psum + 32*bias); exp scale divides by 32^2
    q_b = np.ascontiguousarray(WS * qkv_b[idx_q].reshape(PAIRS, 128).T)
    k_b = np.ascontiguousarray(WS * qkv_b[idx_k].reshape(PAIRS, 128).T)
    pbe = proj_b + proj_w @ qkv_b[idx_v]
    proj_beff = np.ascontiguousarray(pbe.astype(f).reshape(NT, 128).T)

    norm_w_c = np.ascontiguousarray(np.asarray(norm_w, f).reshape(NT, 128).T)
    norm_b_c = np.ascontiguousarray(np.asarray(norm_b, f).reshape(NT, 128).T)

    pp = np.arange(128)
    A_grp = (pp[:, None] // 32 == np.arange(4)[None, :]).astype(f)
    A2T = np.ascontiguousarray(A_grp.T)

    return dict(
        q_w8=q_w8, k_w8=k_w8, v_w8=v_w8, p_w8=p_w8,
        q_b=q_b, k_b=k_b, proj_beff=proj_beff,
        norm_w_c=norm_w_c, norm_b_c=norm_b_c, A_grp=A_grp, A2T=A2T,
    )


def kernel(x, norm_w, norm_b, qkv_w, qkv_b, proj_w, proj_b, _trace=False):
    x = np.asarray(x, np.float32)
    shared = host_pack(norm_w, norm_b, qkv_w, qkv_b, proj_w, proj_b)
    nc = build_program()
    in_maps = [dict(shared, x=np.ascontiguousarray(x[i])) for i in range(B)]
    res = run_bass_kernel_spmd(nc, in_maps, list(range(B)), trace=_trace)
    out = np.stack([res.results[i]["out"] for i in range(B)], axis=0)
    if _trace:
        kernel._last_results = res
    return out.astype(np.float32)
